# revision 70
# baseline (speedup 1.0000x reference)
"""Trainium2 Bass kernel for nn_Attention_loss_919123001759 — symmetric scheme.

Contrastive-style loss:
    H = concat(f1, f2)               [N=8192, D=1024], rows L2-normalized
    e = exp(H @ H.T / t)             [N, N], t = 0.05
    num_i = sum_j e_ij * (S2_ij + eps) * cat_ij
    den_i = sum_j e_ij * negmask_ij  (negmask excludes j=i, |i-j|=B)
    loss = -mean(log(num_i / den_i))

Exploits sim-matrix symmetry: core k (rows R..R+1024, R=1024k) computes only
circulant column blocks d=0..4 (5120 of 8192 cols; fp8 DoubleRow matmuls).
Blocks d=1,2,3 are "mirror" blocks: their transposes (blocks 5,6,7 of other
cores) are recovered with cheap column-reduce matmuls on the PE (one-hot
stationaries -> per-slot partitions of a single psum bank), cutting PE sim
work to 40/64 of the full grid.

Specials (j=i diag in d=0, j=i+-B cross in d=4) are excluded on-chip by a
"poke" matmul: a tiny bf16 MM (Id stationary x -C*Id moving, F=128) adds
-32768 into the psum diagonal of each special tile before the exp, so
e==0 exactly at the excluded positions. Host adds the exact special num
terms in float64. This lets den row sums ride the ACT exp accumulator on
EVERY tile (no DVE masked STTs / deferred-STT juggling).

Mirror den col sums use a second fp8 pair-plane exp + DoubleRow reduce
(halves the PE den-col cost; ACT has headroom), except the final pair,
which reduces bf16 e directly so the tail's last col matmul isn't gated
on the slower epk chain. The col-reduce matmuls for tile m are emitted
after tile m+2's sim matmuls (software pipelining by two tiles, carried
across span boundaries) so the PE never stalls on the psum->ACT exp->DVE
prod round trip. Span order [0,1,4,2,3] ends on a mirror span and keeps
the PE-dense special spans apart (a dense stretch risks the P0 2.0GHz
power downclock); row-sum outputs for the first four spans ship mid-run
so only 32KB rides the tail, in contiguous dram tensors (strided slices
would DMA as 4-byte scatter descriptors, +5us of completion wait).

Hard-won constraints baked in here:
  - GPSIMD/Pool is a trap: it shares SBUF ports with DVE (concurrent pool
    work slows DVE STTs 2.5x), walrus rejects TensorScalarPtr on Pool, and
    PE moving-operand reads of pool-written SBUF fault without drains.
  - matmul start=True clears the whole psum BANK's has_written bits; only
    one start per bank, everything else accumulates via overwrite-on-first-
    touch.
  - a matmul whose stationary and moving operands overlap the same SBUF
    bytes wedges the core -> lhsT is a separate copy of rhs span 0.
  - the HAM power manager gives ~12us of full clock, dips, then ~100us
    sustained; a DMA-dense start trips an early half-clock dip, so the
    fill stays single-queue.
  - SCALAR_TENSOR_TENSOR runs at 1x on DVE (not registered for 2x_1p);
    TENSOR_TENSOR runs 2x with all-2B dtypes.
  - p-major DRAM layouts (8-32KB per-partition contiguous descriptors);
    1KB descriptors measured ~150GB/s aggregate vs ~400GB/s for big ones.
"""

import numpy as np
import ml_dtypes

BF16 = ml_dtypes.bfloat16
FP8 = ml_dtypes.float8_e4m3fn

B = 4096          # batch (rows of f1/f2)
D = 1024          # feature dim
N = 2 * B         # total rows
NCORES = 8
RPC = N // NCORES  # rows per core = 1024
P = 128           # partitions
KC = D // P       # K chunks = 8
MBLK = RPC // P   # row blocks per core = 8
NSPAN = 5         # column blocks d=0..4 (5120 cols)
T_INV = 20.0      # 1/t
EPS_W = 1e-5

FP8_SCALE = 16.0
MM_SCALE = T_INV / (FP8_SCALE * FP8_SCALE)
POKE_C = 32768.0   # psum diag subtrahend: exp(MM_SCALE*(s - C)) == 0

SPECIAL_SPANS = (0, 4)   # d=0 diag, d=4 cross
MIRROR_SPANS = (1, 2, 3)


_NC_CACHE = {}


def _split_sync_waits(nc):
    """Legalize for this walrus build: TPB instruction structs hold only ONE
    inline sync-wait (EventSemaphore: two), so move excess waits onto
    standalone EventSemaphore (wait-only) instructions placed just before, on
    the same engine."""
    import concourse.mybir as mybir

    n_new = 0
    for f in nc.m.functions:
        for b in f.blocks:
            out = []
            changed = False
            for inst in b.instructions:
                si = getattr(inst, "sync_info", None)
                waits = list(si.on_wait) if si and si.on_wait else []
                if len(waits) > 1:
                    excess, keep = waits[:-1], waits[-1:]
                    for i in range(0, len(excess), 2):
                        ev = mybir.InstEventSemaphore(
                            name=f"Wsplit-{n_new}", ins=[], outs=[])
                        ev.engine = inst.engine
                        ev.sync_info = mybir.SyncInfo(
                            on_wait=excess[i:i + 2], on_update=[])
                        out.append(ev)
                        n_new += 1
                    inst.sync_info = mybir.SyncInfo(
                        on_wait=keep, on_update=list(si.on_update))
                    changed = True
                out.append(inst)
            if changed:
                b.instructions = out
    return n_new


def _dedup_ldweights(nc):
    """Drop an InstLdweights whose weights operand is identical to the
    previous one on the PE stream with only matmuls in between."""
    import concourse.mybir as mybir

    removed = 0
    for f in nc.m.functions:
        for b in f.blocks:
            out = []
            last_key = None
            pend_w, pend_u = [], []
            for inst in b.instructions:
                tn = type(inst).__name__
                if getattr(inst, "engine", None) != mybir.EngineType.PE:
                    out.append(inst)
                    continue
                if tn == "InstLdweights":
                    a = inst.ins[0]
                    key = (getattr(a, "memref", None), a.offset, str(a.ap),
                           str(getattr(inst, "perf_mode", None)))
                    si = getattr(inst, "sync_info", None)
                    if key == last_key:
                        if si:
                            pend_w.extend(si.on_wait or [])
                            pend_u.extend(si.on_update or [])
                        removed += 1
                        continue
                    last_key = key
                    out.append(inst)
                elif tn == "InstMatmult":
                    if pend_w or pend_u:
                        si = getattr(inst, "sync_info", None)
                        ow = list(si.on_wait) if si and si.on_wait else []
                        ou = list(si.on_update) if si and si.on_update else []
                        inst.sync_info = mybir.SyncInfo(
                            on_wait=pend_w + ow, on_update=ou + pend_u)
                        pend_w, pend_u = [], []
                    out.append(inst)
                elif tn == "InstEventSemaphore":
                    # sync-only: doesn't touch the weight registers, so keep
                    # the dedup window open across it
                    out.append(inst)
                else:
                    last_key = None
                    if pend_w or pend_u:
                        si = getattr(inst, "sync_info", None)
                        ow = list(si.on_wait) if si and si.on_wait else []
                        ou = list(si.on_update) if si and si.on_update else []
                        inst.sync_info = mybir.SyncInfo(
                            on_wait=pend_w + ow, on_update=ou + pend_u)
                        pend_w, pend_u = [], []
                    out.append(inst)
            assert not pend_w and not pend_u
            b.instructions = out
    return removed


def _build_nc():
    """Build the single SPMD Bass program (identical on all cores)."""
    import concourse.bass as bass
    import concourse.tile as tile
    import concourse.mybir as mybir

    f32 = mybir.dt.float32
    bf16 = mybir.dt.bfloat16
    fp8 = mybir.dt.float8e4
    GS = 2            # k-chunks per matmul (DoubleRow pairs)
    NG = KC // GS     # 4
    DR = mybir.MatmulPerfMode.DoubleRow
    MUL = mybir.AluOpType.mult
    EXP = mybir.ActivationFunctionType.Exp
    COPY = mybir.ActivationFunctionType.Copy

    nc = bass.Bass(trn_type="TRN2", debug=False)

    # p-major DRAM layouts: each partition's data is one long contiguous
    # chunk -> 8-32KB DMA descriptors (1KB descriptors measured ~150GB/s
    # aggregate; big ones run near peak). lhsT is span 0 of rhs (same data).
    rhs_d = nc.dram_tensor("rhs", [P, NSPAN, KC, 1024], fp8,
                           kind="ExternalInput")
    w_d = nc.dram_tensor("w", [P, MBLK, B], fp8, kind="ExternalInput")
    wp_d = nc.dram_tensor("wp", [3, P, MBLK, RPC], bf16,
                          kind="ExternalInput")
    poke_d = nc.dram_tensor("poke", [P, 2, P], bf16, kind="ExternalInput")
    # row-sum outputs split: slots 0-3 = spans processed 1st-4th (d=0,1,4,2),
    # shipped as soon as the 4th span drains; the *_last pair holds the
    # final span's (d=3) sums so only 32KB rides the tail. All contiguous --
    # a strided slice of one tensor would DMA as 4-byte scatter descriptors.
    num_d = nc.dram_tensor("num_out", [P, MBLK, 4], f32,
                           kind="ExternalOutput")
    den_d = nc.dram_tensor("den_out", [P, MBLK, 4], f32,
                           kind="ExternalOutput")
    nd4_d = nc.dram_tensor("nd_last", [P, 2, MBLK], f32,
                           kind="ExternalOutput")
    # col_out[d-1] = [den_h0; den_h1; num_h0; num_h1] rows of 512 column
    # sums each; j index is the span's column order directly.
    col_d = nc.dram_tensor("col_out", [3, 16, 512], f32,
                           kind="ExternalOutput")

    with tile.TileContext(nc) as tc:
        with (
            tc.tile_pool(name="const", bufs=1) as cpool,
            tc.tile_pool(name="rhsp", bufs=3) as rpool,
            tc.tile_pool(name="ep", bufs=10) as epool,
            tc.tile_pool(name="prodp", bufs=3) as prpool,
            tc.tile_pool(name="scrp", bufs=4) as spool,
            tc.tile_pool(name="accp", bufs=1) as apool,
            tc.tile_pool(name="psum", bufs=3, space="PSUM") as pspool,
            tc.tile_pool(name="psumc", bufs=2, space="PSUM") as pcpool,
        ):
            # HAM warmup: dummy matmuls with no DMA deps keep the PE busy
            # through the input-DMA wait (p-state ramp + overlap); count
            # sized to the dual-ring fill time (~3us at cold clock)
            warm = cpool.tile([P, 128], fp8)
            nc.vector.memset(warm[:], 0.0)
            wps = pspool.tile([P, 1024], f32, tag="ps", name="wps")
            for _ in range(32):
                nc.tensor.matmul(wps[:, :128], warm[:], warm[:],
                                 start=True, stop=True)

            # ones stationaries for the column reduces. Each slot's ones sit
            # at a different M position so the four accumulators (den h0/h1,
            # num h0/h1) land on different PARTITIONS of one psum bank.
            CM = 16
            st_num = [cpool.tile([P, CM], bf16, name=f"stnum{j}")
                      for j in range(2)]
            for j in range(2):
                nc.vector.memset(st_num[j][:], 0.0)
                nc.vector.memset(st_num[j][:, j + 2:j + 3], 1.0)
            # fp8 DoubleRow den stationaries (both planes one-hot at col j);
            # M=16: fp8 DR ldweights rejects M<16 (s3_lw_dual_fp8)
            st_den8 = [cpool.tile([P, 2, CM], fp8, name=f"stden8{j}")
                       for j in range(2)]
            for j in range(2):
                nc.vector.memset(st_den8[j][:], 0.0)
                nc.vector.memset(st_den8[j][:, :, j:j + 1], 1.0)
            # bf16 den stationaries for the very last m-pair (its den cols
            # reduce bf16 e directly: e is ready ~1.1us before the epk
            # plane would be, shortening the tail's critical chain)
            st_den = [cpool.tile([P, CM], bf16, name=f"stden{j}")
                      for j in range(2)]
            for j in range(2):
                nc.vector.memset(st_den[j][:], 0.0)
                nc.vector.memset(st_den[j][:, j:j + 1], 1.0)

            # poke constants: Id and -C*Id [P, 128] bf16 (DMA'd; a diagonal
            # is not expressible as an SBUF memset AP)
            poke_sb = cpool.tile([P, 2, P], bf16, name="pokesb")
            id_sb = poke_sb[:, 0]
            negc_sb = poke_sb[:, 1]

            w_sb = cpool.tile([P, MBLK, B], fp8, name="w")
            wp_sb = [cpool.tile([P, MBLK, RPC], bf16, name=f"wp{i}")
                     for i in range(3)]

            # spans 0-3 accumulate into one pair of tiles DMA'd out as soon
            # as span 3 drains; span 4's slots live in tiny separate tiles so
            # only they ride the tail.
            num_parts = apool.tile([P, MBLK, 4], f32)
            den_parts = apool.tile([P, MBLK, 4], f32)
            # one tile -> ONE tail DMA: separate posts would serialize
            # their ~1-2us HBM-write completions on the sync ring
            nd_last = apool.tile([P, 2, MBLK], f32)
            num_last = nd_last[:, 0]
            den_last = nd_last[:, 1]

            # span 0 doubles as the stationary operand for every span, so it
            # lives in a persistent tile; spans 1-4 double-buffer. A separate
            # copy serves as the stationary so no matmul reads the same bytes
            # as both operands.
            span0_sb = cpool.tile([P, KC, 1024], fp8, name="span0")
            lhsT_sb = cpool.tile([P, KC, 1024], fp8, name="lhsTsb")

            def load_span(d):
                if d == 0:
                    t = span0_sb
                else:
                    t = rpool.tile([P, KC, 1024], fp8, tag="rhs",
                                   name=f"rhs_{d}")
                nc.sync.dma_start(t[:], rhs_d[:, d])
                return t

            # DMA order: first-needed first; wp_i rides ahead of the span
            # that needs it next.
            # halves of lhsT/span0 interleaved: early DMA throughput is
            # concurrency-limited, so more in-flight transfers fill faster
            rhs_sb = [None] * NSPAN
            rhs_sb[0] = span0_sb
            # poke first (tiny, needed by the warm-start pokes ~11us), then
            # lhsT/span0 as interleaved chunk-pair pieces: the g-outer warm
            # start below begins real sim work on piece 0 (~512KB) instead
            # of waiting for the full 2MB fill.
            nc.sync.dma_start(poke_sb[:], poke_d[:])
            for g in range(KC // GS):
                sl = slice(g * GS, (g + 1) * GS)
                nc.sync.dma_start(lhsT_sb[:, sl], rhs_d[:, 0, sl])
                nc.sync.dma_start(span0_sb[:, sl], rhs_d[:, 0, sl])
            nc.sync.dma_start(w_sb[:, :MBLK // 2], w_d[:, :MBLK // 2])
            rhs_sb[1] = load_span(1)
            # wp0 halves: the first half covers d1's early TTs ~3us sooner
            nc.sync.dma_start(wp_sb[0][:, :MBLK // 2], wp_d[0, :, :MBLK // 2])
            nc.sync.dma_start(wp_sb[0][:, MBLK // 2:], wp_d[0, :, MBLK // 2:])
            nc.sync.dma_start(w_sb[:, MBLK // 2:], w_d[:, MBLK // 2:])

            def lhsT_slice(g, m):
                return lhsT_sb[:, g * GS:(g + 1) * GS, m * P:(m + 1) * P]

            # col-reduce matmuls for tile m are deferred until after tile
            # m+2's sim matmuls (pipeline by two, across span boundaries):
            # by then the ACT exp -> DVE prod / ACT epk chains for tile m
            # have fully drained, so the PE never waits on them.
            pends = []   # (cp, e, prod, epk, m, dcol)

            def flush_pend():
                cp, e, prod, epk, m, dcol = pends.pop(0)
                last = m == MBLK - 1
                for h in range(2):
                    nc.tensor.matmul(
                        cp[:], st_num[h][:], prod[:, h * 512:(h + 1) * 512],
                        start=(m == 0 and h == 0), stop=last)
                if epk is None:
                    # tail tiles: bf16 den col sums straight from e
                    for h in range(2):
                        nc.tensor.matmul(
                            cp[:], st_den[h][:], e[:, h * 512:(h + 1) * 512],
                            start=False, stop=last)
                elif m % 2 == 1:
                    # den col sums: fp8 DoubleRow over the m-pair's planes
                    for h in range(2):
                        nc.tensor.matmul(
                            cp[:], st_den8[h][:],
                            epk[:, :, h * 512:(h + 1) * 512],
                            start=False, stop=last,
                            perf_mode=DR)
                if last:
                    csb = spool.tile([CM, 512], f32, tag="colsb")
                    nc.scalar.activation(csb[:], cp[:], COPY)
                    # the final span's col DMA posts from the scalar queue,
                    # right behind its COPY, in parallel with the sync ring
                    eng = nc.scalar if dcol == 3 else nc.sync
                    eng.dma_start(col_d[dcol - 1], csb[:])

            SPAN_ORDER = [0, 1, 4, 2, 3]
            ORDER_IDX = {d: i for i, d in enumerate(SPAN_ORDER)}
            early_ps = {}
            for si, d in enumerate(SPAN_ORDER):
                # prefetch upcoming spans / wp just-in-time
                if si == 0:
                    rhs_sb[4] = load_span(4)
                elif si == 1:
                    rhs_sb[2] = load_span(2)
                    nc.sync.dma_start(wp_sb[1][:], wp_d[1])
                elif si == 2:
                    rhs_sb[3] = load_span(3)
                    nc.sync.dma_start(wp_sb[2][:], wp_d[2])

                special = d in SPECIAL_SPANS
                wc0 = (d * 1024) % B
                ps_pre = {}
                if d == 0:
                    # warm start: g-outer over the first 3 tiles so sim work
                    # begins as soon as DMA piece 0 (lhsT/span0 chunks 0-1)
                    # lands, instead of waiting for the whole 2MB fill
                    NPRE = 3
                    for mp in range(NPRE):
                        ps_pre[mp] = pspool.tile([P, 1024], f32, tag="ps",
                                                 name=f"ps_0_{mp}")
                    for g in range(NG):
                        for mp in range(NPRE):
                            for h in range(2):
                                nc.tensor.matmul(
                                    ps_pre[mp][:, h * 512:(h + 1) * 512],
                                    lhsT_slice(g, mp),
                                    rhs_sb[0][:, g * GS:(g + 1) * GS,
                                              h * 512:(h + 1) * 512],
                                    start=(g == 0),
                                    stop=(g == NG - 1),
                                    perf_mode=DR,
                                )
                        if g == 0:
                            for mp in range(NPRE):
                                nc.tensor.matmul(
                                    ps_pre[mp][:, mp * P:(mp + 1) * P],
                                    id_sb[:], negc_sb[:],
                                    start=False, stop=False)
                if not special:
                    # single-bank col accumulator: partitions 0-3 = den h0,
                    # den h1, num h0, num h1. One start=True clears the
                    # bank's has_written bits; every slot's first write then
                    # overwrites (bank init for free), later ones add.
                    cp = pcpool.tile([CM, 512], f32, tag="colacc",
                                     name=f"colacc_{d}")
                for m in range(MBLK):
                    if m in ps_pre:
                        ps = ps_pre[m]   # sims already emitted (warm start)
                    else:
                        if m == MBLK - 1 and si + 1 < len(SPAN_ORDER):
                            # pre-allocate the NEXT span's first psum tile
                            # ahead of this span's last: it then recycles a
                            # buffer freed by an exp two tiles older, so the
                            # span transition carries no psum WAR stall
                            # (~0.8us). Pure allocation-order change -- no
                            # instruction moves, every engine queue is
                            # untouched (unlike the rejected v11/v12).
                            d_n = SPAN_ORDER[si + 1]
                            early_ps[d_n] = pspool.tile(
                                [P, 1024], f32, tag="ps",
                                name=f"ps_early_{d_n}")
                        ps = early_ps.pop(d, None) if m == 0 else None
                        if ps is None:
                            ps = pspool.tile([P, 1024], f32, tag="ps",
                                             name=f"ps_{d}_{m}")
                        for g in range(NG):
                            for h in range(2):
                                nc.tensor.matmul(
                                    ps[:, h * 512:(h + 1) * 512],
                                    lhsT_slice(g, m),
                                    rhs_sb[d][:, g * GS:(g + 1) * GS,
                                              h * 512:(h + 1) * 512],
                                    start=(g == 0),
                                    stop=(g == NG - 1),
                                    perf_mode=DR,
                                )
                            if g == 0 and special:
                                # poke: ps[q, m*128+q] += -C so the excluded
                                # position (diag for d=0, cross for d=4) exps
                                # to exactly 0. Must follow the g==0 pair
                                # (bank has_written clear) and precede stop.
                                nc.tensor.matmul(
                                    ps[:, m * P:(m + 1) * P],
                                    id_sb[:], negc_sb[:],
                                    start=False, stop=False)
                    if pends and (len(pends) >= 2
                                  or (pends[0][5] != d and m >= 2)):
                        flush_pend()
                    oi = ORDER_IDX[d]
                    den_acc = (den_last[:, m:m + 1] if oi == 4
                               else den_parts[:, m, oi:oi + 1])
                    num_acc = (num_last[:, m:m + 1] if oi == 4
                               else num_parts[:, m, oi:oi + 1])
                    e = epool.tile([P, 1024], bf16, tag="e")
                    # den row sum rides the ACT accumulator on every tile
                    nc.scalar.activation(
                        e[:], ps[:], EXP, scale=MM_SCALE,
                        accum_out=den_acc,
                    )
                    if not special:
                        # second exp -> fp8 pair plane for the DoubleRow
                        # den-col reduce (ACT has headroom; keeps bf16 e
                        # for the fast DVE product and precise num rows).
                        # Every span's last pair skips it and reduces bf16 e
                        # directly: mirror spans are ACT-paced (2494ns/tile
                        # vs PE 2376), so dropping the last pair's epk exps
                        # drains the ACT drift before the span ends -- the
                        # next span's first sims otherwise stall ~0.7us on a
                        # psum buffer still pending its lagging exp. Also
                        # shortens the final tail chain.
                        skip_epk = m >= MBLK - 2
                        if skip_epk:
                            epk = None
                        else:
                            if m % 2 == 0:
                                epk = prpool.tile([P, 2, 1024], fp8,
                                                  tag="epk")
                            nc.scalar.activation(epk[:, m % 2, :], ps[:],
                                                 EXP, scale=MM_SCALE)
                        prod = prpool.tile([P, 1024], bf16, tag="prod")
                        nc.vector.tensor_tensor(
                            out=prod[:],
                            in0=e[:],
                            in1=wp_sb[d - 1][:, m, :],
                            op=MUL,
                        )
                        pends.append((cp, e, prod, epk, m, d))
                    # num row sum (DVE)
                    scr2 = spool.tile([P, 1024], bf16, tag="scr")
                    nc.vector.scalar_tensor_tensor(
                        out=scr2[:],
                        in0=e[:],
                        scalar=1.0,
                        in1=w_sb[:, m, wc0:wc0 + 1024],
                        op0=MUL,
                        op1=MUL,
                        accum_out=num_acc,
                    )
                if si == 3:
                    # first four spans fully accumulated: ship the big
                    # accumulator tiles now so only the last span's tiny
                    # slices ride the tail
                    nc.sync.dma_start(num_d[:], num_parts[:])
                    nc.sync.dma_start(den_d[:], den_parts[:])
            while pends:
                flush_pend()

            # final small outputs split across the two HW-DGE queues so the
            # ~600ns ring posts don't fully serialize in the tail
            nc.sync.dma_start(nd4_d[:], nd_last[:], single_packet=True)

    _dedup_ldweights(nc)
    _split_sync_waits(nc)
    return nc


def get_nc():
    if "nc" not in _NC_CACHE:
        _NC_CACHE["nc"] = _build_nc()
    return _NC_CACHE["nc"]


def prep_inputs(feature1, feature2, S_weight, pre_label):
    """Build the 8 per-core input maps + host-side special terms."""
    f1 = np.ascontiguousarray(np.asarray(feature1, dtype=np.float32))
    f2 = np.ascontiguousarray(np.asarray(feature2, dtype=np.float32))
    S = np.asarray(S_weight, dtype=np.float32)
    labels = np.asarray(pre_label).astype(np.int64)

    H = np.concatenate([f1, f2], axis=0)            # [N, D] f32
    HT = np.ascontiguousarray(H.T)                  # [D, N] f32
    HT_q = (HT * np.float32(FP8_SCALE)).astype(FP8)

    poke = np.zeros((P, 2, P), dtype=np.float32)
    poke[:, 0, :] = np.eye(P, dtype=np.float32)
    poke[:, 1, :] = -POKE_C * np.eye(P, dtype=np.float32)
    poke = poke.astype(BF16)

    in_maps = []
    for k in range(NCORES):
        R = RPC * k
        rho = R % B
        # full circulant: rotated col c <-> global col (R + c) mod N.
        cols = (R + np.arange(NSPAN * 1024)) % N
        # [P, NSPAN, KC, 1024]: per-partition contiguous 40KB
        rhs = np.ascontiguousarray(
            HT_q[:, cols].reshape(KC, P, NSPAN, 1024)
            .transpose(1, 2, 0, 3))
        # w for direct num rows: B-periodic, so only cols [0, B) needed.
        perm = (np.arange(B) + rho) % B
        rows = np.arange(rho, rho + RPC)
        Sp = S[rows][:, perm] + np.float32(EPS_W)   # [1024, 4096]
        cat = labels[rows][:, None] == labels[perm][None, :]
        w = np.where(cat, Sp, np.float32(0.0))
        ii = np.arange(RPC)
        w[ii, ii] = 0.0   # kills both d=0 diag and d=4 cross in num
        # w' for mirror cols: wp[d-1][r, c'] = w_full[R+1024d+c', R+r]
        wps = []
        for d in MIRROR_SPANS:
            jr = (rho + 1024 * d + np.arange(RPC)) % B
            Mjr = (S[np.ix_(jr, rows)] + np.float32(EPS_W)) * (
                labels[jr][:, None] == labels[rows][None, :])
            wps.append(Mjr.T)                        # [r, c']
        # [3, P, MBLK, RPC] p-major
        wp = np.ascontiguousarray(
            np.stack(wps, axis=0).astype(BF16)
            .reshape(3, MBLK, P, RPC).transpose(0, 2, 1, 3))
        # [P, MBLK, B] p-major
        w_pm = np.ascontiguousarray(
            w.astype(FP8).reshape(MBLK, P, B).transpose(1, 0, 2))
        in_maps.append({
            "rhs": rhs,
            "w": w_pm,
            "wp": wp,
            "poke": poke,
        })

    # host-side special terms in float64
    H64 = H.astype(np.float64)
    sim_ii = np.einsum("ij,ij->i", H64, H64)            # [N] ~ 1.0
    cross = np.einsum("ij,ij->i", f1.astype(np.float64),
                      f2.astype(np.float64))            # [B]
    e_ii = np.exp(sim_ii * T_INV)
    e_cross = np.exp(np.concatenate([cross, cross]) * T_INV)
    s_ii = S.diagonal().astype(np.float64)
    s_cross = np.concatenate([s_ii, s_ii])
    num_special = EPS_W * e_ii + (s_cross + EPS_W) * e_cross  # [N]
    return in_maps, num_special


def postprocess(results, num_special):
    # num_out/den_out hold spans processed 1st-4th; *_last the 5th (d=3)
    num = np.concatenate(
        [(np.asarray(r["num_out"], dtype=np.float64).sum(-1)
          + np.asarray(r["nd_last"], dtype=np.float64)[:, 0]).T.reshape(-1)
         for r in results])
    den = np.concatenate(
        [(np.asarray(r["den_out"], dtype=np.float64).sum(-1)
          + np.asarray(r["nd_last"], dtype=np.float64)[:, 1]).T.reshape(-1)
         for r in results])
    # mirror col contributions: core k block d covers global rows
    # (1024k + 1024d .. +1024), local j = q*128 + p
    for k in range(NCORES):
        R = RPC * k
        col = np.asarray(results[k]["col_out"], dtype=np.float64)  # [3,4,512]
        for i, d in enumerate(MIRROR_SPANS):
            j0 = (R + 1024 * d) % N
            den[j0:j0 + RPC] += col[i, 0:2].reshape(-1)
            num[j0:j0 + RPC] += col[i, 2:4].reshape(-1)
    num_total = num + num_special
    loss = -np.mean(np.log(num_total / den))
    return np.float32(loss)


def kernel(feature1, feature2, S_weight, pre_label):
    from concourse.bass_utils import run_bass_kernel_spmd

    nc = get_nc()
    in_maps, num_special = prep_inputs(feature1, feature2, S_weight, pre_label)
    res = run_bass_kernel_spmd(nc, in_maps, core_ids=list(range(NCORES)))
    return postprocess(res.results, num_special)


# revision 71
# speedup vs baseline: 1.0067x; 1.0067x over previous
"""Trainium2 Bass kernel for nn_Attention_loss_919123001759 — symmetric scheme.

Contrastive-style loss:
    H = concat(f1, f2)               [N=8192, D=1024], rows L2-normalized
    e = exp(H @ H.T / t)             [N, N], t = 0.05
    num_i = sum_j e_ij * (S2_ij + eps) * cat_ij
    den_i = sum_j e_ij * negmask_ij  (negmask excludes j=i, |i-j|=B)
    loss = -mean(log(num_i / den_i))

Exploits sim-matrix symmetry: core k (rows R..R+1024, R=1024k) computes only
circulant column blocks d=0..4 (5120 of 8192 cols; fp8 DoubleRow matmuls).
Blocks d=1,2,3 are "mirror" blocks: their transposes (blocks 5,6,7 of other
cores) are recovered with cheap column-reduce matmuls on the PE (one-hot
stationaries -> per-slot partitions of a single psum bank), cutting PE sim
work to 40/64 of the full grid.

Specials (j=i diag in d=0, j=i+-B cross in d=4) are excluded on-chip by a
"poke" matmul: a tiny bf16 MM (Id stationary x -C*Id moving, F=128) adds
-32768 into the psum diagonal of each special tile before the exp, so
e==0 exactly at the excluded positions. Host adds the exact special num
terms in float64. This lets den row sums ride the ACT exp accumulator on
EVERY tile (no DVE masked STTs / deferred-STT juggling).

Mirror den col sums use a second fp8 pair-plane exp + DoubleRow reduce
(halves the PE den-col cost; ACT has headroom), except the final pair,
which reduces bf16 e directly so the tail's last col matmul isn't gated
on the slower epk chain. The col-reduce matmuls for tile m are emitted
after tile m+2's sim matmuls (software pipelining by two tiles, carried
across span boundaries) so the PE never stalls on the psum->ACT exp->DVE
prod round trip. Span order [0,1,4,2,3] ends on a mirror span and keeps
the PE-dense special spans apart (a dense stretch risks the P0 2.0GHz
power downclock); row-sum outputs for the first four spans ship mid-run
so only 32KB rides the tail, in contiguous dram tensors (strided slices
would DMA as 4-byte scatter descriptors, +5us of completion wait).

Hard-won constraints baked in here:
  - GPSIMD/Pool is a trap: it shares SBUF ports with DVE (concurrent pool
    work slows DVE STTs 2.5x), walrus rejects TensorScalarPtr on Pool, and
    PE moving-operand reads of pool-written SBUF fault without drains.
  - matmul start=True clears the whole psum BANK's has_written bits; only
    one start per bank, everything else accumulates via overwrite-on-first-
    touch.
  - a matmul whose stationary and moving operands overlap the same SBUF
    bytes wedges the core -> lhsT is a separate copy of rhs span 0.
  - the HAM power manager gives ~12us of full clock, dips, then ~100us
    sustained; a DMA-dense start trips an early half-clock dip, so the
    fill stays single-queue.
  - SCALAR_TENSOR_TENSOR runs at 1x on DVE (not registered for 2x_1p);
    TENSOR_TENSOR runs 2x with all-2B dtypes.
  - p-major DRAM layouts (8-32KB per-partition contiguous descriptors);
    1KB descriptors measured ~150GB/s aggregate vs ~400GB/s for big ones.
"""

import numpy as np
import ml_dtypes

BF16 = ml_dtypes.bfloat16
FP8 = ml_dtypes.float8_e4m3fn

B = 4096          # batch (rows of f1/f2)
D = 1024          # feature dim
N = 2 * B         # total rows
NCORES = 8
RPC = N // NCORES  # rows per core = 1024
P = 128           # partitions
KC = D // P       # K chunks = 8
MBLK = RPC // P   # row blocks per core = 8
NSPAN = 5         # column blocks d=0..4 (5120 cols)
T_INV = 20.0      # 1/t
EPS_W = 1e-5

FP8_SCALE = 16.0
MM_SCALE = T_INV / (FP8_SCALE * FP8_SCALE)
POKE_C = 32768.0   # psum diag subtrahend: exp(MM_SCALE*(s - C)) == 0

SPECIAL_SPANS = (0, 4)   # d=0 diag, d=4 cross
MIRROR_SPANS = (1, 2, 3)


_NC_CACHE = {}


def _split_sync_waits(nc):
    """Legalize for this walrus build: TPB instruction structs hold only ONE
    inline sync-wait (EventSemaphore: two), so move excess waits onto
    standalone EventSemaphore (wait-only) instructions placed just before, on
    the same engine."""
    import concourse.mybir as mybir

    n_new = 0
    for f in nc.m.functions:
        for b in f.blocks:
            out = []
            changed = False
            for inst in b.instructions:
                si = getattr(inst, "sync_info", None)
                waits = list(si.on_wait) if si and si.on_wait else []
                if len(waits) > 1:
                    excess, keep = waits[:-1], waits[-1:]
                    for i in range(0, len(excess), 2):
                        ev = mybir.InstEventSemaphore(
                            name=f"Wsplit-{n_new}", ins=[], outs=[])
                        ev.engine = inst.engine
                        ev.sync_info = mybir.SyncInfo(
                            on_wait=excess[i:i + 2], on_update=[])
                        out.append(ev)
                        n_new += 1
                    inst.sync_info = mybir.SyncInfo(
                        on_wait=keep, on_update=list(si.on_update))
                    changed = True
                out.append(inst)
            if changed:
                b.instructions = out
    return n_new


def _dedup_ldweights(nc):
    """Drop an InstLdweights whose weights operand is identical to the
    previous one on the PE stream with only matmuls in between."""
    import concourse.mybir as mybir

    removed = 0
    for f in nc.m.functions:
        for b in f.blocks:
            out = []
            last_key = None
            pend_w, pend_u = [], []
            for inst in b.instructions:
                tn = type(inst).__name__
                if getattr(inst, "engine", None) != mybir.EngineType.PE:
                    out.append(inst)
                    continue
                if tn == "InstLdweights":
                    a = inst.ins[0]
                    key = (getattr(a, "memref", None), a.offset, str(a.ap),
                           str(getattr(inst, "perf_mode", None)))
                    si = getattr(inst, "sync_info", None)
                    if key == last_key:
                        if si:
                            pend_w.extend(si.on_wait or [])
                            pend_u.extend(si.on_update or [])
                        removed += 1
                        continue
                    last_key = key
                    out.append(inst)
                elif tn == "InstMatmult":
                    if pend_w or pend_u:
                        si = getattr(inst, "sync_info", None)
                        ow = list(si.on_wait) if si and si.on_wait else []
                        ou = list(si.on_update) if si and si.on_update else []
                        inst.sync_info = mybir.SyncInfo(
                            on_wait=pend_w + ow, on_update=ou + pend_u)
                        pend_w, pend_u = [], []
                    out.append(inst)
                elif tn == "InstEventSemaphore":
                    # sync-only: doesn't touch the weight registers, so keep
                    # the dedup window open across it
                    out.append(inst)
                else:
                    last_key = None
                    if pend_w or pend_u:
                        si = getattr(inst, "sync_info", None)
                        ow = list(si.on_wait) if si and si.on_wait else []
                        ou = list(si.on_update) if si and si.on_update else []
                        inst.sync_info = mybir.SyncInfo(
                            on_wait=pend_w + ow, on_update=ou + pend_u)
                        pend_w, pend_u = [], []
                    out.append(inst)
            assert not pend_w and not pend_u
            b.instructions = out
    return removed


def _build_nc():
    """Build the single SPMD Bass program (identical on all cores)."""
    import concourse.bass as bass
    import concourse.tile as tile
    import concourse.mybir as mybir

    f32 = mybir.dt.float32
    bf16 = mybir.dt.bfloat16
    fp8 = mybir.dt.float8e4
    GS = 2            # k-chunks per matmul (DoubleRow pairs)
    NG = KC // GS     # 4
    DR = mybir.MatmulPerfMode.DoubleRow
    MUL = mybir.AluOpType.mult
    EXP = mybir.ActivationFunctionType.Exp
    COPY = mybir.ActivationFunctionType.Copy

    nc = bass.Bass(trn_type="TRN2", debug=False)

    # p-major DRAM layouts: each partition's data is one long contiguous
    # chunk -> 8-32KB DMA descriptors (1KB descriptors measured ~150GB/s
    # aggregate; big ones run near peak). lhsT is span 0 of rhs (same data).
    rhs_d = nc.dram_tensor("rhs", [P, NSPAN, KC, 1024], fp8,
                           kind="ExternalInput")
    w_d = nc.dram_tensor("w", [P, MBLK, B], fp8, kind="ExternalInput")
    wp_d = nc.dram_tensor("wp", [3, P, MBLK, RPC], bf16,
                          kind="ExternalInput")
    poke_d = nc.dram_tensor("poke", [P, 2, P], bf16, kind="ExternalInput")
    # row-sum outputs split: slots 0-3 = spans processed 1st-4th (d=0,1,4,2),
    # shipped as soon as the 4th span drains; the *_last pair holds the
    # final span's (d=3) sums so only 32KB rides the tail. All contiguous --
    # a strided slice of one tensor would DMA as 4-byte scatter descriptors.
    num_d = nc.dram_tensor("num_out", [P, MBLK, 4], f32,
                           kind="ExternalOutput")
    den_d = nc.dram_tensor("den_out", [P, MBLK, 4], f32,
                           kind="ExternalOutput")
    nd4_d = nc.dram_tensor("nd_last", [P, 2, MBLK], f32,
                           kind="ExternalOutput")
    # col_out[d-1] = [den_h0; den_h1; num_h0; num_h1] rows of 512 column
    # sums each; j index is the span's column order directly.
    col_d = nc.dram_tensor("col_out", [3, 16, 512], f32,
                           kind="ExternalOutput")

    with tile.TileContext(nc) as tc:
        with (
            tc.tile_pool(name="const", bufs=1) as cpool,
            tc.tile_pool(name="rhsp", bufs=3) as rpool,
            tc.tile_pool(name="ep", bufs=10) as epool,
            tc.tile_pool(name="prodp", bufs=3) as prpool,
            tc.tile_pool(name="scrp", bufs=4) as spool,
            tc.tile_pool(name="accp", bufs=1) as apool,
            tc.tile_pool(name="psum", bufs=3, space="PSUM") as pspool,
            tc.tile_pool(name="psumc", bufs=2, space="PSUM") as pcpool,
        ):
            # HAM warmup: dummy matmuls with no DMA deps keep the PE busy
            # through the input-DMA wait (p-state ramp + overlap); count
            # sized to the dual-ring fill time (~3us at cold clock)
            warm = cpool.tile([P, 128], fp8)
            nc.vector.memset(warm[:], 0.0)
            wps = pspool.tile([P, 1024], f32, tag="ps", name="wps")
            for _ in range(32):
                nc.tensor.matmul(wps[:, :128], warm[:], warm[:],
                                 start=True, stop=True)

            # ones stationaries for the column reduces. Each slot's ones sit
            # at a different M position so the four accumulators (den h0/h1,
            # num h0/h1) land on different PARTITIONS of one psum bank.
            CM = 16
            st_num = [cpool.tile([P, CM], bf16, name=f"stnum{j}")
                      for j in range(2)]
            for j in range(2):
                nc.vector.memset(st_num[j][:], 0.0)
                nc.vector.memset(st_num[j][:, j + 2:j + 3], 1.0)
            # fp8 DoubleRow den stationaries (both planes one-hot at col j);
            # M=16: fp8 DR ldweights rejects M<16 (s3_lw_dual_fp8)
            st_den8 = [cpool.tile([P, 2, CM], fp8, name=f"stden8{j}")
                       for j in range(2)]
            for j in range(2):
                nc.vector.memset(st_den8[j][:], 0.0)
                nc.vector.memset(st_den8[j][:, :, j:j + 1], 1.0)
            # bf16 den stationaries for the very last m-pair (its den cols
            # reduce bf16 e directly: e is ready ~1.1us before the epk
            # plane would be, shortening the tail's critical chain)
            st_den = [cpool.tile([P, CM], bf16, name=f"stden{j}")
                      for j in range(2)]
            for j in range(2):
                nc.vector.memset(st_den[j][:], 0.0)
                nc.vector.memset(st_den[j][:, j:j + 1], 1.0)

            # poke constants: Id and -C*Id [P, 128] bf16 (DMA'd; a diagonal
            # is not expressible as an SBUF memset AP)
            poke_sb = cpool.tile([P, 2, P], bf16, name="pokesb")
            id_sb = poke_sb[:, 0]
            negc_sb = poke_sb[:, 1]

            w_sb = cpool.tile([P, MBLK, B], fp8, name="w")
            wp_sb = [cpool.tile([P, MBLK, RPC], bf16, name=f"wp{i}")
                     for i in range(3)]

            # spans 0-3 accumulate into one pair of tiles DMA'd out as soon
            # as span 3 drains; span 4's slots live in tiny separate tiles so
            # only they ride the tail.
            num_parts = apool.tile([P, MBLK, 4], f32)
            den_parts = apool.tile([P, MBLK, 4], f32)
            # one tile -> ONE tail DMA: separate posts would serialize
            # their ~1-2us HBM-write completions on the sync ring
            nd_last = apool.tile([P, 2, MBLK], f32)
            num_last = nd_last[:, 0]
            den_last = nd_last[:, 1]

            # span 0 doubles as the stationary operand for every span, so it
            # lives in a persistent tile; spans 1-4 double-buffer. A separate
            # copy serves as the stationary so no matmul reads the same bytes
            # as both operands.
            span0_sb = cpool.tile([P, KC, 1024], fp8, name="span0")
            lhsT_sb = cpool.tile([P, KC, 1024], fp8, name="lhsTsb")

            def load_span(d):
                if d == 0:
                    t = span0_sb
                else:
                    t = rpool.tile([P, KC, 1024], fp8, tag="rhs",
                                   name=f"rhs_{d}")
                nc.sync.dma_start(t[:], rhs_d[:, d])
                return t

            # DMA order: first-needed first; wp_i rides ahead of the span
            # that needs it next.
            # halves of lhsT/span0 interleaved: early DMA throughput is
            # concurrency-limited, so more in-flight transfers fill faster
            rhs_sb = [None] * NSPAN
            rhs_sb[0] = span0_sb
            # poke first (tiny, needed by the warm-start pokes ~11us), then
            # lhsT/span0 as interleaved chunk-pair pieces: the g-outer warm
            # start below begins real sim work on piece 0 (~512KB) instead
            # of waiting for the full 2MB fill.
            nc.sync.dma_start(poke_sb[:], poke_d[:])
            for g in range(KC // GS):
                sl = slice(g * GS, (g + 1) * GS)
                nc.sync.dma_start(lhsT_sb[:, sl], rhs_d[:, 0, sl])
                nc.sync.dma_start(span0_sb[:, sl], rhs_d[:, 0, sl])
            nc.sync.dma_start(w_sb[:, :MBLK // 2], w_d[:, :MBLK // 2])
            rhs_sb[1] = load_span(1)
            # wp0 halves: the first half covers d1's early TTs ~3us sooner
            nc.sync.dma_start(wp_sb[0][:, :MBLK // 2], wp_d[0, :, :MBLK // 2])
            nc.sync.dma_start(wp_sb[0][:, MBLK // 2:], wp_d[0, :, MBLK // 2:])
            nc.sync.dma_start(w_sb[:, MBLK // 2:], w_d[:, MBLK // 2:])

            def lhsT_slice(g, m):
                return lhsT_sb[:, g * GS:(g + 1) * GS, m * P:(m + 1) * P]

            # col-reduce matmuls for tile m are deferred until after tile
            # m+2's sim matmuls (pipeline by two, across span boundaries):
            # by then the ACT exp -> DVE prod / ACT epk chains for tile m
            # have fully drained, so the PE never waits on them.
            pends = []   # (cp, e, prod, epk, m, dcol)

            def flush_pend():
                cp, e, prod, epk, m, dcol = pends.pop(0)
                last = m == MBLK - 1
                for h in range(2):
                    nc.tensor.matmul(
                        cp[:], st_num[h][:], prod[:, h * 512:(h + 1) * 512],
                        start=(m == 0 and h == 0), stop=last)
                if epk is None:
                    # tail tiles: bf16 den col sums straight from e
                    for h in range(2):
                        nc.tensor.matmul(
                            cp[:], st_den[h][:], e[:, h * 512:(h + 1) * 512],
                            start=False, stop=last)
                elif m % 2 == 1:
                    # den col sums: fp8 DoubleRow over the m-pair's planes
                    for h in range(2):
                        nc.tensor.matmul(
                            cp[:], st_den8[h][:],
                            epk[:, :, h * 512:(h + 1) * 512],
                            start=False, stop=last,
                            perf_mode=DR)
                if last:
                    csb = spool.tile([CM, 512], f32, tag="colsb")
                    nc.scalar.activation(csb[:], cp[:], COPY)
                    # the final span's col DMA posts from the scalar queue,
                    # right behind its COPY, in parallel with the sync ring
                    eng = nc.scalar if dcol == 3 else nc.sync
                    eng.dma_start(col_d[dcol - 1], csb[:])

            SPAN_ORDER = [0, 1, 4, 2, 3]
            ORDER_IDX = {d: i for i, d in enumerate(SPAN_ORDER)}
            for si, d in enumerate(SPAN_ORDER):
                # prefetch upcoming spans / wp just-in-time
                if si == 0:
                    rhs_sb[4] = load_span(4)
                elif si == 1:
                    rhs_sb[2] = load_span(2)
                    nc.sync.dma_start(wp_sb[1][:], wp_d[1])
                elif si == 2:
                    rhs_sb[3] = load_span(3)
                    nc.sync.dma_start(wp_sb[2][:], wp_d[2])

                special = d in SPECIAL_SPANS
                wc0 = (d * 1024) % B
                ps_pre = {}
                if d == 0:
                    # warm start: g-outer over the first 3 tiles so sim work
                    # begins as soon as DMA piece 0 (lhsT/span0 chunks 0-1)
                    # lands, instead of waiting for the whole 2MB fill
                    NPRE = 3
                    for mp in range(NPRE):
                        ps_pre[mp] = pspool.tile([P, 1024], f32, tag="ps",
                                                 name=f"ps_0_{mp}")
                    for g in range(NG):
                        for mp in range(NPRE):
                            for h in range(2):
                                nc.tensor.matmul(
                                    ps_pre[mp][:, h * 512:(h + 1) * 512],
                                    lhsT_slice(g, mp),
                                    rhs_sb[0][:, g * GS:(g + 1) * GS,
                                              h * 512:(h + 1) * 512],
                                    start=(g == 0),
                                    stop=(g == NG - 1),
                                    perf_mode=DR,
                                )
                        if g == 0:
                            for mp in range(NPRE):
                                nc.tensor.matmul(
                                    ps_pre[mp][:, mp * P:(mp + 1) * P],
                                    id_sb[:], negc_sb[:],
                                    start=False, stop=False)
                if not special:
                    # single-bank col accumulator: partitions 0-3 = den h0,
                    # den h1, num h0, num h1. One start=True clears the
                    # bank's has_written bits; every slot's first write then
                    # overwrites (bank init for free), later ones add.
                    cp = pcpool.tile([CM, 512], f32, tag="colacc",
                                     name=f"colacc_{d}")
                for m in range(MBLK):
                    if m in ps_pre:
                        ps = ps_pre[m]   # sims already emitted (warm start)
                    else:
                        ps = pspool.tile([P, 1024], f32, tag="ps",
                                         name=f"ps_{d}_{m}")
                        for g in range(NG):
                            for h in range(2):
                                nc.tensor.matmul(
                                    ps[:, h * 512:(h + 1) * 512],
                                    lhsT_slice(g, m),
                                    rhs_sb[d][:, g * GS:(g + 1) * GS,
                                              h * 512:(h + 1) * 512],
                                    start=(g == 0),
                                    stop=(g == NG - 1),
                                    perf_mode=DR,
                                )
                            if g == 0 and special:
                                # poke: ps[q, m*128+q] += -C so the excluded
                                # position (diag for d=0, cross for d=4) exps
                                # to exactly 0. Must follow the g==0 pair
                                # (bank has_written clear) and precede stop.
                                nc.tensor.matmul(
                                    ps[:, m * P:(m + 1) * P],
                                    id_sb[:], negc_sb[:],
                                    start=False, stop=False)
                    if pends and (len(pends) >= 2
                                  or (pends[0][5] != d and m >= 2)):
                        flush_pend()
                    oi = ORDER_IDX[d]
                    den_acc = (den_last[:, m:m + 1] if oi == 4
                               else den_parts[:, m, oi:oi + 1])
                    num_acc = (num_last[:, m:m + 1] if oi == 4
                               else num_parts[:, m, oi:oi + 1])
                    e = epool.tile([P, 1024], bf16, tag="e")
                    # den row sum rides the ACT accumulator on every tile
                    nc.scalar.activation(
                        e[:], ps[:], EXP, scale=MM_SCALE,
                        accum_out=den_acc,
                    )
                    if not special:
                        # second exp -> fp8 pair plane for the DoubleRow
                        # den-col reduce (ACT has headroom; keeps bf16 e
                        # for the fast DVE product and precise num rows).
                        # Every span's last pair skips it and reduces bf16 e
                        # directly: mirror spans are ACT-paced (2494ns/tile
                        # vs PE 2376), so dropping the last pair's epk exps
                        # drains the ACT drift before the span ends -- the
                        # next span's first sims otherwise stall ~0.7us on a
                        # psum buffer still pending its lagging exp. Also
                        # shortens the final tail chain.
                        skip_epk = m >= MBLK - 2
                        if skip_epk:
                            epk = None
                        else:
                            if m % 2 == 0:
                                epk = prpool.tile([P, 2, 1024], fp8,
                                                  tag="epk")
                            nc.scalar.activation(epk[:, m % 2, :], ps[:],
                                                 EXP, scale=MM_SCALE)
                        prod = prpool.tile([P, 1024], bf16, tag="prod")
                        nc.vector.tensor_tensor(
                            out=prod[:],
                            in0=e[:],
                            in1=wp_sb[d - 1][:, m, :],
                            op=MUL,
                        )
                        pends.append((cp, e, prod, epk, m, d))
                    # num row sum (DVE)
                    scr2 = spool.tile([P, 1024], bf16, tag="scr")
                    nc.vector.scalar_tensor_tensor(
                        out=scr2[:],
                        in0=e[:],
                        scalar=1.0,
                        in1=w_sb[:, m, wc0:wc0 + 1024],
                        op0=MUL,
                        op1=MUL,
                        accum_out=num_acc,
                    )
                if si == 3:
                    # first four spans fully accumulated: ship the big
                    # accumulator tiles now so only the last span's tiny
                    # slices ride the tail
                    nc.sync.dma_start(num_d[:], num_parts[:])
                    nc.sync.dma_start(den_d[:], den_parts[:])
            while pends:
                flush_pend()

            # final small outputs split across the two HW-DGE queues so the
            # ~600ns ring posts don't fully serialize in the tail
            nc.sync.dma_start(nd4_d[:], nd_last[:], single_packet=True)

    _dedup_ldweights(nc)
    _split_sync_waits(nc)
    return nc


def get_nc():
    if "nc" not in _NC_CACHE:
        _NC_CACHE["nc"] = _build_nc()
    return _NC_CACHE["nc"]


def prep_inputs(feature1, feature2, S_weight, pre_label):
    """Build the 8 per-core input maps + host-side special terms."""
    f1 = np.ascontiguousarray(np.asarray(feature1, dtype=np.float32))
    f2 = np.ascontiguousarray(np.asarray(feature2, dtype=np.float32))
    S = np.asarray(S_weight, dtype=np.float32)
    labels = np.asarray(pre_label).astype(np.int64)

    H = np.concatenate([f1, f2], axis=0)            # [N, D] f32
    HT = np.ascontiguousarray(H.T)                  # [D, N] f32
    HT_q = (HT * np.float32(FP8_SCALE)).astype(FP8)

    poke = np.zeros((P, 2, P), dtype=np.float32)
    poke[:, 0, :] = np.eye(P, dtype=np.float32)
    poke[:, 1, :] = -POKE_C * np.eye(P, dtype=np.float32)
    poke = poke.astype(BF16)

    in_maps = []
    for k in range(NCORES):
        R = RPC * k
        rho = R % B
        # full circulant: rotated col c <-> global col (R + c) mod N.
        cols = (R + np.arange(NSPAN * 1024)) % N
        # [P, NSPAN, KC, 1024]: per-partition contiguous 40KB
        rhs = np.ascontiguousarray(
            HT_q[:, cols].reshape(KC, P, NSPAN, 1024)
            .transpose(1, 2, 0, 3))
        # w for direct num rows: B-periodic, so only cols [0, B) needed.
        perm = (np.arange(B) + rho) % B
        rows = np.arange(rho, rho + RPC)
        Sp = S[rows][:, perm] + np.float32(EPS_W)   # [1024, 4096]
        cat = labels[rows][:, None] == labels[perm][None, :]
        w = np.where(cat, Sp, np.float32(0.0))
        ii = np.arange(RPC)
        w[ii, ii] = 0.0   # kills both d=0 diag and d=4 cross in num
        # w' for mirror cols: wp[d-1][r, c'] = w_full[R+1024d+c', R+r]
        wps = []
        for d in MIRROR_SPANS:
            jr = (rho + 1024 * d + np.arange(RPC)) % B
            Mjr = (S[np.ix_(jr, rows)] + np.float32(EPS_W)) * (
                labels[jr][:, None] == labels[rows][None, :])
            wps.append(Mjr.T)                        # [r, c']
        # [3, P, MBLK, RPC] p-major
        wp = np.ascontiguousarray(
            np.stack(wps, axis=0).astype(BF16)
            .reshape(3, MBLK, P, RPC).transpose(0, 2, 1, 3))
        # [P, MBLK, B] p-major
        w_pm = np.ascontiguousarray(
            w.astype(FP8).reshape(MBLK, P, B).transpose(1, 0, 2))
        in_maps.append({
            "rhs": rhs,
            "w": w_pm,
            "wp": wp,
            "poke": poke,
        })

    # host-side special terms in float64
    H64 = H.astype(np.float64)
    sim_ii = np.einsum("ij,ij->i", H64, H64)            # [N] ~ 1.0
    cross = np.einsum("ij,ij->i", f1.astype(np.float64),
                      f2.astype(np.float64))            # [B]
    e_ii = np.exp(sim_ii * T_INV)
    e_cross = np.exp(np.concatenate([cross, cross]) * T_INV)
    s_ii = S.diagonal().astype(np.float64)
    s_cross = np.concatenate([s_ii, s_ii])
    num_special = EPS_W * e_ii + (s_cross + EPS_W) * e_cross  # [N]
    return in_maps, num_special


def postprocess(results, num_special):
    # num_out/den_out hold spans processed 1st-4th; *_last the 5th (d=3)
    num = np.concatenate(
        [(np.asarray(r["num_out"], dtype=np.float64).sum(-1)
          + np.asarray(r["nd_last"], dtype=np.float64)[:, 0]).T.reshape(-1)
         for r in results])
    den = np.concatenate(
        [(np.asarray(r["den_out"], dtype=np.float64).sum(-1)
          + np.asarray(r["nd_last"], dtype=np.float64)[:, 1]).T.reshape(-1)
         for r in results])
    # mirror col contributions: core k block d covers global rows
    # (1024k + 1024d .. +1024), local j = q*128 + p
    for k in range(NCORES):
        R = RPC * k
        col = np.asarray(results[k]["col_out"], dtype=np.float64)  # [3,4,512]
        for i, d in enumerate(MIRROR_SPANS):
            j0 = (R + 1024 * d) % N
            den[j0:j0 + RPC] += col[i, 0:2].reshape(-1)
            num[j0:j0 + RPC] += col[i, 2:4].reshape(-1)
    num_total = num + num_special
    loss = -np.mean(np.log(num_total / den))
    return np.float32(loss)


def kernel(feature1, feature2, S_weight, pre_label):
    from concourse.bass_utils import run_bass_kernel_spmd

    nc = get_nc()
    in_maps, num_special = prep_inputs(feature1, feature2, S_weight, pre_label)
    res = run_bass_kernel_spmd(nc, in_maps, core_ids=list(range(NCORES)))
    return postprocess(res.results, num_special)
